# revision 16
# baseline (speedup 1.0000x reference)
"""Trainium2 Bass kernel for ContextualGatingCollapse (linear attention, one query per batch).

Math (per batch b, head h, phi(z) = elu(z)+1 = min(exp(z),1) + relu(z)):
    phiq = phi(x @ Wq + bq)                     [1, 1024]   (host)
    k    = y @ Wk                               [S, 1024]   (PE, fp8 DoubleRow)
    w[s,h] = sum_{d in block_h} phi(k)[s,d] * phiq[d]       (ACT exp/relu + DVE stt/mul/reduce)
    u[h,:] = sum_s w[s,h] * y[s,:]              [16, 1024]  (PE, fp32 PSUM)
    den[h] = sum_s w[s,h]                                    (PE, ones column in the u-acc matmul)
    ctx[block_h] = (u[h,:] / (den[h]+eps)) @ Wv[:, block_h]  (PE finale)
    out  = ctx @ Wo + bo                                     (PE finale + host bias)

Never computes the V projection: each head's numerator only reads head-block
columns of V, so num[h, block_h] = (sum_s w[s,h] y[s,:]) @ Wv[:, block_h].
The q path runs on host.

Engine balance per 128-row subtile (measured-model ns, both near-saturated):
    PE   : 8 DR matmuls (kp) + 3 deferred u/den matmuls    ~2800
    ACT  : exp + relu from PSUM                            ~2100
    DVE  : stt combine + phiq mul + per-head reduce        ~2900
GPSIMD/Pool offload was tried and is a dead end: its bf16 tensor ops run far
below the DVE on real hardware.  fp16 for the u-acc matmul operands was also
tried: fp16 matmuls are slower than bf16 on the PE.

K projection dtype: fp8-e4m3 with DoubleRow (2 MACs/cell/cycle).  Wk is
pre-scaled by 32 (exact power of 2) so fp8 operands have unit variance; the
ACT affine (scale=1/32) undoes it before exp/relu.  All accumulation fp32 PSUM.

Sharding: data-parallel over batch, 2 batches per NeuronCore x 8 cores.
"""

import os
import sys

import numpy as np

for _p in ("/opt/trn_rl_repo", "/root/.axon_site/_ro/trn_rl_repo"):
    if os.path.isdir(_p) and _p not in sys.path:
        sys.path.insert(0, _p)

import ml_dtypes
from contextlib import ExitStack

import concourse.bass as bass
import concourse.tile as tile
from concourse import bacc, mybir
from concourse.bass_utils import run_bass_kernel_spmd

B, S, D, H, HD = 16, 4096, 1024, 16, 64
NCORES = 8
BPC = B // NCORES      # 2 batches per core
EPS = 1e-6
FP = mybir.dt.float32
BF = mybir.dt.bfloat16
F16 = mybir.dt.float16
F8 = mybir.dt.float8e4
SCHUNK = 512           # s-chunk per DMA tile
NSC = S // SCHUNK      # 8
NSUB = SCHUNK // 128   # 4 psum subtiles per s-chunk
NDC = D // 128         # 8 contraction chunks (bf16)
NCC = D // 256         # 4 contraction chunks (fp8 DoubleRow)
WKSCALE = 32.0
UDELAY = 3             # u-acc deferral depth (subtiles)
Exp = mybir.ActivationFunctionType.Exp
Relu = mybir.ActivationFunctionType.Relu
Add = mybir.AluOpType.add
Min = mybir.AluOpType.min
Mult = mybir.AluOpType.mult
AxX = mybir.AxisListType.X
AxC = mybir.AxisListType.C
DR = mybir.MatmulPerfMode.DoubleRow


def _emit_u(nc, acc, onescol_f16, pending, last):
    """u/den accumulation for one (deferred) subtile: u += w0r.T @ y_sd,
    den += w0r.T @ ones.  Emitted UDELAY subtiles late so the PE never waits
    on the phi chain."""
    w0r, ys, j, first = pending
    nc.tensor.matmul(acc[:, 0:512], lhsT=w0r[:], rhs=ys[:, j, 0:512],
                     start=first, stop=last)
    nc.tensor.matmul(acc[:, 512:1024], lhsT=w0r[:], rhs=ys[:, j, 512:1024],
                     start=first, stop=last)
    nc.tensor.matmul(acc[:, 1024:1026], lhsT=w0r[:], rhs=onescol_f16[:],
                     start=first, stop=last)


def _build(nc: bass.Bass, repeat: int = 1):
    yk_d = nc.dram_tensor("yk", [BPC, NSC, 128, NCC, 2, SCHUNK], F8,
                          kind="ExternalInput")
    wk_d = nc.dram_tensor("wk", [128, NCC, 2, D], F8, kind="ExternalInput")
    ysd_d = nc.dram_tensor("ysd", [BPC, S, D], BF, kind="ExternalInput")
    phiq_d = nc.dram_tensor("phiqr", [BPC, 128, D], BF, kind="ExternalInput")
    wv_d = nc.dram_tensor("wv", [128, NDC, D], BF, kind="ExternalInput")
    wo_d = nc.dram_tensor("wo", [128, NDC, D], BF, kind="ExternalInput")
    out_d = nc.dram_tensor("out", [BPC, D], FP, kind="ExternalOutput")

    # Small constants embedded in the NEFF.
    onescol_d = nc.inline_tensor(np.ones((128, 2), np.float32), "onescol")
    eye16_d = nc.inline_tensor(np.eye(H, dtype=np.float32), "eye16")
    ident1_d = nc.inline_tensor(np.ones((1, 1), np.float32), "ident1")
    ones16_d = nc.inline_tensor(np.ones((H, 1), np.float32), "ones16")
    mask_np = np.zeros((H, D), np.float32)
    for h in range(H):
        mask_np[h, h * HD:(h + 1) * HD] = 1.0
    mask_d = nc.inline_tensor(mask_np, "maskhd")

    with tile.TileContext(nc) as tc, ExitStack() as ctx:
        wpool = ctx.enter_context(tc.tile_pool(name="wpool", bufs=1))
        cpool = ctx.enter_context(tc.tile_pool(name="cpool", bufs=1))
        # PSUM: 2 rotating kp tiles (2 banks each) + one 3-bank accumulator
        # (u in cols 0..1023, den in 1024..1025).
        kpool = ctx.enter_context(
            tc.tile_pool(name="kpool", bufs=2, space=bass.MemorySpace.PSUM))
        accps = ctx.enter_context(
            tc.tile_pool(name="accps", bufs=1, space=bass.MemorySpace.PSUM))

        # ---- weights / constants needed by the main loop ----
        wk_sb = wpool.tile([128, NCC, 2, D], F8, tag="wk")
        nc.sync.dma_start(wk_sb[:], wk_d[:])
        phiq_sb = wpool.tile([128, BPC, D], BF, tag="phiq")
        nc.sync.dma_start(phiq_sb[:], phiq_d[:].rearrange("b p d -> p b d"))

        # One-time all-engine sync so steady-state instructions don't carry
        # per-weight-DMA waits (walrus caps sync-wait commands per inst).
        tc.strict_bb_all_engine_barrier()

        # Finale-only tensors: issued after the barrier so they load during
        # the main loop instead of lengthening startup.
        wv_sb = wpool.tile([128, NDC, D], BF, tag="wv")
        nc.sync.dma_start(wv_sb[:], wv_d[:])
        wo_sb = wpool.tile([128, NDC, D], BF, tag="wo")
        nc.sync.dma_start(wo_sb[:], wo_d[:])
        mask_sb = cpool.tile([H, D], FP, tag="mask")
        nc.sync.dma_start(mask_sb[:], mask_d[:])
        eye16_sb = cpool.tile([H, H], FP, tag="eye16")
        nc.sync.dma_start(eye16_sb[:], eye16_d[:])
        ident1_sb = cpool.tile([1, 1], FP, tag="ident1")
        nc.sync.dma_start(ident1_sb[:], ident1_d[:])
        ones16_sb = cpool.tile([H, 1], FP, tag="ones16")
        nc.sync.dma_start(ones16_sb[:], ones16_d[:])
        ones16_bf = cpool.tile([H, 1], BF, tag="ones16bf")
        nc.vector.tensor_copy(ones16_bf[:], ones16_sb[:])
        onescol_sb = cpool.tile([128, 2], FP, tag="onescol")
        nc.sync.dma_start(onescol_sb[:], onescol_d[:])
        onescol_f16 = cpool.tile([128, 2], BF, tag="onescolf16")
        nc.vector.tensor_copy(onescol_f16[:], onescol_sb[:])

        ykpool = ctx.enter_context(tc.tile_pool(name="ykpool", bufs=3))
        yspool = ctx.enter_context(tc.tile_pool(name="yspool", bufs=3))
        work = ctx.enter_context(tc.tile_pool(name="work", bufs=2))
        lpool = ctx.enter_context(tc.tile_pool(name="lpool", bufs=UDELAY + 2))
        upool = ctx.enter_context(tc.tile_pool(name="upool", bufs=1))

        # Optional in-kernel repetition (timing only).
        rep_cm = tc.For_i(0, repeat, 1) if repeat > 1 else None
        if rep_cm is not None:
            rep_cm.__enter__()

        u_sb = [upool.tile([H, D + 2], FP, tag=f"u{b}", name=f"u{b}")
                for b in range(BPC)]

        for b in range(BPC):
            acc = accps.tile([H, D + 2], FP, tag="acc")
            pend = []
            sub = 0
            for sc in range(NSC):
                yk = ykpool.tile([128, NCC, 2, SCHUNK], F8, tag="yk")
                nc.sync.dma_start(
                    yk[:], yk_d[b:b + 1, sc:sc + 1].rearrange(
                        "o t p c i s -> (o t p) c i s"))
                ys = yspool.tile([128, NSUB, D], BF, tag="ys")
                nc.sync.dma_start(
                    ys[:], ysd_d[b:b + 1, sc * SCHUNK:(sc + 1) * SCHUNK, :]
                    .rearrange("o (j p) d -> (o p) j d", p=128))
                for j in range(NSUB):
                    kp = kpool.tile([128, D], FP, tag="kp")
                    for cc in range(NCC):
                        fl, ll = cc == 0, cc == NCC - 1
                        lt = yk[:, cc, :, bass.ts(j, 128)]
                        nc.tensor.matmul(kp[:, 0:512], lhsT=lt,
                                         rhs=wk_sb[:, cc, :, 0:512],
                                         start=fl, stop=ll, perf_mode=DR)
                        nc.tensor.matmul(kp[:, 512:1024], lhsT=lt,
                                         rhs=wk_sb[:, cc, :, 512:1024],
                                         start=fl, stop=ll, perf_mode=DR)
                    # phi(k) = min(exp(k),1) + relu(k); ACT affine undoes the
                    # host-side Wk*32 scaling.  exp/relu on ACT; fused
                    # min+add (stt), phiq mul, and per-head reduce (straight
                    # to bf16, single rounding) on DVE.
                    e_t = work.tile([128, D], BF, tag="e")
                    nc.scalar.activation(e_t[:], kp[:], Exp, scale=1.0 / WKSCALE)
                    r_t = work.tile([128, D], BF, tag="r")
                    nc.scalar.activation(r_t[:], kp[:], Relu, scale=1.0 / WKSCALE)
                    pk = work.tile([128, D], BF, tag="pk")
                    nc.vector.scalar_tensor_tensor(pk[:], e_t[:], 1.0, r_t[:],
                                                   Min, Add)
                    pkq = work.tile([128, D], BF, tag="pkq")
                    nc.vector.tensor_mul(pkq[:], pk[:], phiq_sb[:, b, :])
                    w0r = lpool.tile([128, H], BF, tag="w0r")
                    with nc.allow_low_precision(reason="single rounding to "
                                                "bf16 for the u-acc lhsT"):
                        nc.vector.tensor_reduce(
                            w0r[:], pkq[:].rearrange("p (h d) -> p h d", h=H),
                            axis=AxX, op=Add)
                    pend.append((w0r, ys, j, sub == 0))
                    if len(pend) > UDELAY:
                        _emit_u(nc, acc, onescol_f16, pend.pop(0), last=False)
                    sub += 1
            for i, p in enumerate(pend):
                _emit_u(nc, acc, onescol_f16, p, last=(i == len(pend) - 1))
            pend = []
            nc.vector.tensor_copy(u_sb[b][:], acc[:])

        # ---- finale: ctx[block_h] = (u[h,:] @ Wv[:, block_h]) / den[h],
        # out = ctx @ Wo.  The 1/den row scale commutes through Wv, so raw u
        # is transposed (PE) and rcp folds into the mask stt on the drain.
        usT = cpool.tile([128, NDC, BPC, H], BF, tag="usT")
        ctx_rows = [cpool.tile([1, D], FP, tag=f"ctxrow{b}", name=f"ctxrow{b}")
                    for b in range(BPC)]
        for b in range(BPC):
            for c in range(NDC):
                tp = kpool.tile([128, H], FP, tag="kp")
                nc.tensor.transpose(tp[:], u_sb[b][:, bass.ts(c, 128)],
                                    eye16_sb[:])
                nc.scalar.copy(usT[:, c, b, :], tp[:])
            dsb = cpool.tile([H, 1], FP, tag=f"dsb{b}")
            nc.vector.tensor_scalar_add(dsb[:], u_sb[b][:, D:D + 1], EPS)
            rcp = cpool.tile([H, 1], FP, tag=f"rcp{b}")
            nc.vector.reciprocal(rcp[:], dsb[:])
            cs = accps.tile([H, D], FP, tag="acc")
            for n in range(2):
                for c in range(NDC):
                    nc.tensor.matmul(cs[:, bass.ts(n, 512)],
                                     lhsT=usT[:, c, b, :],
                                     rhs=wv_sb[:, c, bass.ts(n, 512)],
                                     start=(c == 0), stop=(c == NDC - 1))
            # (cs * 1/den) * head-block mask, fused in one DVE op
            csm = cpool.tile([H, D], BF, tag=f"csm{b}")
            nc.vector.scalar_tensor_tensor(csm[:], cs[:], rcp[:], mask_sb[:],
                                           Mult, Mult)
            for n in range(2):
                cr = kpool.tile([1, 512], FP, tag="kp")
                nc.tensor.matmul(cr[:], lhsT=ones16_bf[:],
                                 rhs=csm[:, bass.ts(n, 512)],
                                 start=True, stop=True)
                nc.scalar.copy(ctx_rows[b][:, bass.ts(n, 512)], cr[:])
        ctxT = cpool.tile([128, NDC, BPC], BF, tag="ctxT")
        for c in range(NDC):
            for b in range(BPC):
                tp = kpool.tile([128, 1], FP, tag="kp")
                nc.tensor.transpose(tp[:], ctx_rows[b][:, bass.ts(c, 128)],
                                    ident1_sb[:])
                nc.scalar.copy(ctxT[:, c, b:b + 1], tp[:])
        out_sb = cpool.tile([BPC, D], FP, tag="outsb")
        for n in range(2):
            op = accps.tile([BPC, 512], FP, tag="acc")
            for c in range(NDC):
                nc.tensor.matmul(op[:], lhsT=ctxT[:, c, :],
                                 rhs=wo_sb[:, c, bass.ts(n, 512)],
                                 start=(c == 0), stop=(c == NDC - 1))
            nc.scalar.copy(out_sb[:, bass.ts(n, 512)], op[:])
        nc.sync.dma_start(out_d[:], out_sb[:])

        if rep_cm is not None:
            rep_cm.__exit__(None, None, None)

    return nc


def prepare(inputs, repeat: int = 1):
    """Build + bacc-compile the program and the per-core input maps."""
    y = np.asarray(inputs["y_superposed"], np.float32)
    x = np.asarray(inputs["x_context"], np.float32)
    Wq = np.asarray(inputs["Wq"], np.float32)
    bq = np.asarray(inputs["bq"], np.float32).reshape(1, D)
    Wk = np.asarray(inputs["Wk"], np.float32)
    bk = np.asarray(inputs["bk"], np.float32)
    Wv = np.asarray(inputs["Wv"], np.float32)
    bv = np.asarray(inputs["bv"], np.float32)
    Wo = np.asarray(inputs["Wo"], np.float32)
    bo = np.asarray(inputs["bo"], np.float32).reshape(1, D)
    assert not np.any(bk) and not np.any(bv), "nonzero bk/bv not supported"

    # q path on host: phiq = elu(x@Wq + bq) + 1
    q = x @ Wq + bq
    phiq = np.where(q > 0, q + 1.0, np.exp(np.minimum(q, 0.0))).astype(np.float32)

    bf = ml_dtypes.bfloat16
    f8 = mybir.dt.np(mybir.dt.float8e4)
    Wk32 = Wk * WKSCALE
    wk_p = np.ascontiguousarray(
        Wk32.reshape(NCC, 2, 128, D).transpose(2, 0, 1, 3)).astype(f8)
    wv_p = np.ascontiguousarray(
        Wv.reshape(NDC, 128, D).transpose(1, 0, 2)).astype(bf)
    wo_p = np.ascontiguousarray(
        Wo.reshape(NDC, 128, D).transpose(1, 0, 2)).astype(bf)

    nc = bacc.Bacc("TRN2", target_bir_lowering=False, debug=False,
                   num_devices=NCORES)
    _build(nc, repeat=repeat)
    nc.compile()

    in_maps = []
    for i in range(NCORES):
        sl = slice(i * BPC, (i + 1) * BPC)
        ysl = y[sl]
        yt = ysl.transpose(0, 2, 1)  # [BPC, D, S]
        yk = np.ascontiguousarray(
            yt.reshape(BPC, NCC, 2, 128, NSC, SCHUNK)
            .transpose(0, 4, 3, 1, 2, 5)).astype(f8)
        m = {
            "yk": yk,
            "ysd": ysl.astype(bf),
            "phiqr": np.ascontiguousarray(
                np.broadcast_to(phiq[sl][:, None, :], (BPC, 128, D))).astype(bf),
            "wk": wk_p,
            "wv": wv_p,
            "wo": wo_p,
        }
        in_maps.append(m)
    return nc, in_maps


def run(inputs, trace=False):
    """Build, compile, and execute on 8 NeuronCores. Returns (out, results)."""
    nc, in_maps = prepare(inputs)
    res = run_bass_kernel_spmd(nc, in_maps, list(range(NCORES)), trace=trace)
    bo = np.asarray(inputs["bo"], np.float32).reshape(1, D)
    out = np.concatenate([r["out"] for r in res.results], axis=0) + bo
    return np.ascontiguousarray(out.astype(np.float32)), res


def kernel(**inputs) -> np.ndarray:
    out, _ = run(inputs, trace=False)
    return out


# revision 17
# speedup vs baseline: 1.0216x; 1.0216x over previous
"""Trainium2 Bass kernel for ContextualGatingCollapse (linear attention, one query per batch).

Math (per batch b, head h, phi(z) = elu(z)+1 = min(exp(z),1) + relu(z)):
    phiq = phi(x @ Wq + bq)                     [1, 1024]   (host)
    k    = y @ Wk                               [S, 1024]   (PE, fp8 DoubleRow)
    w[s,h] = sum_{d in block_h} phi(k)[s,d] * phiq[d]       (ACT exp/relu + DVE stt/mul/reduce)
    u[h,:] = sum_s w[s,h] * y[s,:]              [16, 1024]  (PE, fp32 PSUM)
    den[h] = sum_s w[s,h]                                    (PE, ones column in the u-acc matmul)
    ctx[block_h] = (u[h,:] / (den[h]+eps)) @ Wv[:, block_h]  (PE finale)
    out  = ctx @ Wo + bo                                     (PE finale + host bias)

Never computes the V projection: each head's numerator only reads head-block
columns of V, so num[h, block_h] = (sum_s w[s,h] y[s,:]) @ Wv[:, block_h].
The q path runs on host.

Engine balance per 128-row subtile (measured-model ns, both near-saturated):
    PE   : 8 DR matmuls (kp) + 3 deferred u/den matmuls    ~2800
    ACT  : exp + relu from PSUM                            ~2100
    DVE  : stt combine + phiq mul + per-head reduce        ~2900
GPSIMD/Pool offload was tried and is a dead end: its bf16 tensor ops run far
below the DVE on real hardware.  fp16 for the u-acc matmul operands was also
tried: fp16 matmuls are slower than bf16 on the PE.

K projection dtype: fp8-e4m3 with DoubleRow (2 MACs/cell/cycle).  Wk is
pre-scaled by 32 (exact power of 2) so fp8 operands have unit variance; the
ACT affine (scale=1/32) undoes it before exp/relu.  All accumulation fp32 PSUM.

Sharding: data-parallel over batch, 2 batches per NeuronCore x 8 cores.
"""

import os
import sys

import numpy as np

for _p in ("/opt/trn_rl_repo", "/root/.axon_site/_ro/trn_rl_repo"):
    if os.path.isdir(_p) and _p not in sys.path:
        sys.path.insert(0, _p)

import ml_dtypes
from contextlib import ExitStack

import concourse.bass as bass
import concourse.tile as tile
from concourse import bacc, mybir
from concourse.bass_utils import run_bass_kernel_spmd

B, S, D, H, HD = 16, 4096, 1024, 16, 64
NCORES = 8
BPC = B // NCORES      # 2 batches per core
EPS = 1e-6
FP = mybir.dt.float32
BF = mybir.dt.bfloat16
F16 = mybir.dt.float16
F8 = mybir.dt.float8e4
SCHUNK = 512           # s-chunk per DMA tile
NSC = S // SCHUNK      # 8
NSUB = SCHUNK // 128   # 4 psum subtiles per s-chunk
NDC = D // 128         # 8 contraction chunks (bf16)
NCC = D // 256         # 4 contraction chunks (fp8 DoubleRow)
WKSCALE = 32.0
UDELAY = 2             # u-acc deferral depth (subtiles)
Exp = mybir.ActivationFunctionType.Exp
Relu = mybir.ActivationFunctionType.Relu
Add = mybir.AluOpType.add
Min = mybir.AluOpType.min
Mult = mybir.AluOpType.mult
AxX = mybir.AxisListType.X
AxC = mybir.AxisListType.C
DR = mybir.MatmulPerfMode.DoubleRow


def _emit_u(nc, acc, onescol_f16, pending, last):
    """u/den accumulation for one (deferred) subtile: u += w0r.T @ y_sd,
    den += w0r.T @ ones.  Emitted UDELAY subtiles late so the PE never waits
    on the phi chain."""
    w0r, ys, j, first = pending
    nc.tensor.matmul(acc[:, 0:512], lhsT=w0r[:], rhs=ys[:, j, 0:512],
                     start=first, stop=last)
    nc.tensor.matmul(acc[:, 512:1024], lhsT=w0r[:], rhs=ys[:, j, 512:1024],
                     start=first, stop=last)
    nc.tensor.matmul(acc[:, 1024:1026], lhsT=w0r[:], rhs=onescol_f16[:],
                     start=first, stop=last)


def _build(nc: bass.Bass, repeat: int = 1):
    yk_d = nc.dram_tensor("yk", [BPC, NSC, 128, NCC, 2, SCHUNK], F8,
                          kind="ExternalInput")
    wk_d = nc.dram_tensor("wk", [128, NCC, 2, D], F8, kind="ExternalInput")
    ysd_d = nc.dram_tensor("ysd", [BPC, S, D], BF, kind="ExternalInput")
    phiq_d = nc.dram_tensor("phiqr", [BPC, 128, D], BF, kind="ExternalInput")
    wv_d = nc.dram_tensor("wv", [128, NDC, D], BF, kind="ExternalInput")
    wo_d = nc.dram_tensor("wo", [128, NDC, D], BF, kind="ExternalInput")
    out_d = nc.dram_tensor("out", [BPC, D], FP, kind="ExternalOutput")

    # Small constants embedded in the NEFF.
    onescol_d = nc.inline_tensor(np.ones((128, 2), np.float32), "onescol")
    eye16_d = nc.inline_tensor(np.eye(H, dtype=np.float32), "eye16")
    ident1_d = nc.inline_tensor(np.ones((1, 1), np.float32), "ident1")
    ones16_d = nc.inline_tensor(np.ones((H, 1), np.float32), "ones16")
    mask_np = np.zeros((H, D), np.float32)
    for h in range(H):
        mask_np[h, h * HD:(h + 1) * HD] = 1.0
    mask_d = nc.inline_tensor(mask_np, "maskhd")

    with tile.TileContext(nc) as tc, ExitStack() as ctx:
        wpool = ctx.enter_context(tc.tile_pool(name="wpool", bufs=1))
        cpool = ctx.enter_context(tc.tile_pool(name="cpool", bufs=1))
        # PSUM: 2 rotating kp tiles (2 banks each) + one 3-bank accumulator
        # (u in cols 0..1023, den in 1024..1025).
        kpool = ctx.enter_context(
            tc.tile_pool(name="kpool", bufs=2, space=bass.MemorySpace.PSUM))
        accps = ctx.enter_context(
            tc.tile_pool(name="accps", bufs=1, space=bass.MemorySpace.PSUM))

        # ---- weights / constants needed by the main loop ----
        wk_sb = wpool.tile([128, NCC, 2, D], F8, tag="wk")
        nc.sync.dma_start(wk_sb[:], wk_d[:])
        phiq_sb = wpool.tile([128, BPC, D], BF, tag="phiq")
        nc.sync.dma_start(phiq_sb[:], phiq_d[:].rearrange("b p d -> p b d"))

        # One-time all-engine sync so steady-state instructions don't carry
        # per-weight-DMA waits (walrus caps sync-wait commands per inst).
        tc.strict_bb_all_engine_barrier()

        # Finale-only tensors: issued after the barrier so they load during
        # the main loop instead of lengthening startup.
        wv_sb = wpool.tile([128, NDC, D], BF, tag="wv")
        nc.sync.dma_start(wv_sb[:], wv_d[:])
        wo_sb = wpool.tile([128, NDC, D], BF, tag="wo")
        nc.sync.dma_start(wo_sb[:], wo_d[:])
        mask_sb = cpool.tile([H, D], FP, tag="mask")
        nc.sync.dma_start(mask_sb[:], mask_d[:])
        eye16_sb = cpool.tile([H, H], FP, tag="eye16")
        nc.sync.dma_start(eye16_sb[:], eye16_d[:])
        ident1_sb = cpool.tile([1, 1], FP, tag="ident1")
        nc.sync.dma_start(ident1_sb[:], ident1_d[:])
        ones16_sb = cpool.tile([H, 1], FP, tag="ones16")
        nc.sync.dma_start(ones16_sb[:], ones16_d[:])
        ones16_bf = cpool.tile([H, 1], BF, tag="ones16bf")
        nc.vector.tensor_copy(ones16_bf[:], ones16_sb[:])
        onescol_sb = cpool.tile([128, 2], FP, tag="onescol")
        nc.sync.dma_start(onescol_sb[:], onescol_d[:])
        onescol_f16 = cpool.tile([128, 2], BF, tag="onescolf16")
        nc.vector.tensor_copy(onescol_f16[:], onescol_sb[:])

        ykpool = ctx.enter_context(tc.tile_pool(name="ykpool", bufs=3))
        yspool = ctx.enter_context(tc.tile_pool(name="yspool", bufs=3))
        work = ctx.enter_context(tc.tile_pool(name="work", bufs=2))
        lpool = ctx.enter_context(tc.tile_pool(name="lpool", bufs=UDELAY + 2))
        upool = ctx.enter_context(tc.tile_pool(name="upool", bufs=1))

        # Optional in-kernel repetition (timing only).
        rep_cm = tc.For_i(0, repeat, 1) if repeat > 1 else None
        if rep_cm is not None:
            rep_cm.__enter__()

        u_sb = [upool.tile([H, D + 2], FP, tag=f"u{b}", name=f"u{b}")
                for b in range(BPC)]

        for b in range(BPC):
            acc = accps.tile([H, D + 2], FP, tag="acc")
            pend = []
            sub = 0
            for sc in range(NSC):
                yk = ykpool.tile([128, NCC, 2, SCHUNK], F8, tag="yk")
                nc.sync.dma_start(
                    yk[:], yk_d[b:b + 1, sc:sc + 1].rearrange(
                        "o t p c i s -> (o t p) c i s"))
                ys = yspool.tile([128, NSUB, D], BF, tag="ys")
                nc.sync.dma_start(
                    ys[:], ysd_d[b:b + 1, sc * SCHUNK:(sc + 1) * SCHUNK, :]
                    .rearrange("o (j p) d -> (o p) j d", p=128))
                for j in range(NSUB):
                    kp = kpool.tile([128, D], FP, tag="kp")
                    for cc in range(NCC):
                        fl, ll = cc == 0, cc == NCC - 1
                        lt = yk[:, cc, :, bass.ts(j, 128)]
                        nc.tensor.matmul(kp[:, 0:512], lhsT=lt,
                                         rhs=wk_sb[:, cc, :, 0:512],
                                         start=fl, stop=ll, perf_mode=DR)
                        nc.tensor.matmul(kp[:, 512:1024], lhsT=lt,
                                         rhs=wk_sb[:, cc, :, 512:1024],
                                         start=fl, stop=ll, perf_mode=DR)
                    # phi(k) = min(exp(k),1) + relu(k); ACT affine undoes the
                    # host-side Wk*32 scaling.  exp/relu on ACT; fused
                    # min+add (stt), phiq mul, and per-head reduce (straight
                    # to bf16, single rounding) on DVE.
                    e_t = work.tile([128, D], BF, tag="e")
                    nc.scalar.activation(e_t[:], kp[:], Exp, scale=1.0 / WKSCALE)
                    r_t = work.tile([128, D], BF, tag="r")
                    nc.scalar.activation(r_t[:], kp[:], Relu, scale=1.0 / WKSCALE)
                    pk = work.tile([128, D], BF, tag="pk")
                    nc.vector.scalar_tensor_tensor(pk[:], e_t[:], 1.0, r_t[:],
                                                   Min, Add)
                    pkq = work.tile([128, D], BF, tag="pkq")
                    nc.vector.tensor_mul(pkq[:], pk[:], phiq_sb[:, b, :])
                    w0r = lpool.tile([128, H], BF, tag="w0r")
                    with nc.allow_low_precision(reason="single rounding to "
                                                "bf16 for the u-acc lhsT"):
                        nc.vector.tensor_reduce(
                            w0r[:], pkq[:].rearrange("p (h d) -> p h d", h=H),
                            axis=AxX, op=Add)
                    pend.append((w0r, ys, j, sub == 0))
                    if len(pend) > UDELAY:
                        _emit_u(nc, acc, onescol_f16, pend.pop(0), last=False)
                    sub += 1
            for i, p in enumerate(pend):
                _emit_u(nc, acc, onescol_f16, p, last=(i == len(pend) - 1))
            pend = []
            nc.vector.tensor_copy(u_sb[b][:], acc[:])

        # ---- finale: ctx[block_h] = (u[h,:] @ Wv[:, block_h]) / den[h],
        # out = ctx @ Wo.  The 1/den row scale commutes through Wv, so raw u
        # is transposed (PE) and rcp folds into the mask stt on the drain.
        usT = cpool.tile([128, NDC, BPC, H], BF, tag="usT")
        ctx_rows = [cpool.tile([1, D], FP, tag=f"ctxrow{b}", name=f"ctxrow{b}")
                    for b in range(BPC)]
        for b in range(BPC):
            for c in range(NDC):
                tp = kpool.tile([128, H], FP, tag="kp")
                nc.tensor.transpose(tp[:], u_sb[b][:, bass.ts(c, 128)],
                                    eye16_sb[:])
                nc.scalar.copy(usT[:, c, b, :], tp[:])
            dsb = cpool.tile([H, 1], FP, tag=f"dsb{b}")
            nc.vector.tensor_scalar_add(dsb[:], u_sb[b][:, D:D + 1], EPS)
            rcp = cpool.tile([H, 1], FP, tag=f"rcp{b}")
            nc.vector.reciprocal(rcp[:], dsb[:])
            cs = accps.tile([H, D], FP, tag="acc")
            for n in range(2):
                for c in range(NDC):
                    nc.tensor.matmul(cs[:, bass.ts(n, 512)],
                                     lhsT=usT[:, c, b, :],
                                     rhs=wv_sb[:, c, bass.ts(n, 512)],
                                     start=(c == 0), stop=(c == NDC - 1))
            # (cs * 1/den) * head-block mask, fused in one DVE op
            csm = cpool.tile([H, D], BF, tag=f"csm{b}")
            nc.vector.scalar_tensor_tensor(csm[:], cs[:], rcp[:], mask_sb[:],
                                           Mult, Mult)
            for n in range(2):
                cr = kpool.tile([1, 512], FP, tag="kp")
                nc.tensor.matmul(cr[:], lhsT=ones16_bf[:],
                                 rhs=csm[:, bass.ts(n, 512)],
                                 start=True, stop=True)
                nc.scalar.copy(ctx_rows[b][:, bass.ts(n, 512)], cr[:])
        ctxT = cpool.tile([128, NDC, BPC], BF, tag="ctxT")
        for c in range(NDC):
            for b in range(BPC):
                tp = kpool.tile([128, 1], FP, tag="kp")
                nc.tensor.transpose(tp[:], ctx_rows[b][:, bass.ts(c, 128)],
                                    ident1_sb[:])
                nc.scalar.copy(ctxT[:, c, b:b + 1], tp[:])
        out_sb = cpool.tile([BPC, D], FP, tag="outsb")
        for n in range(2):
            op = accps.tile([BPC, 512], FP, tag="acc")
            for c in range(NDC):
                nc.tensor.matmul(op[:], lhsT=ctxT[:, c, :],
                                 rhs=wo_sb[:, c, bass.ts(n, 512)],
                                 start=(c == 0), stop=(c == NDC - 1))
            nc.scalar.copy(out_sb[:, bass.ts(n, 512)], op[:])
        nc.sync.dma_start(out_d[:], out_sb[:])

        if rep_cm is not None:
            rep_cm.__exit__(None, None, None)

    return nc


def prepare(inputs, repeat: int = 1):
    """Build + bacc-compile the program and the per-core input maps."""
    y = np.asarray(inputs["y_superposed"], np.float32)
    x = np.asarray(inputs["x_context"], np.float32)
    Wq = np.asarray(inputs["Wq"], np.float32)
    bq = np.asarray(inputs["bq"], np.float32).reshape(1, D)
    Wk = np.asarray(inputs["Wk"], np.float32)
    bk = np.asarray(inputs["bk"], np.float32)
    Wv = np.asarray(inputs["Wv"], np.float32)
    bv = np.asarray(inputs["bv"], np.float32)
    Wo = np.asarray(inputs["Wo"], np.float32)
    bo = np.asarray(inputs["bo"], np.float32).reshape(1, D)
    assert not np.any(bk) and not np.any(bv), "nonzero bk/bv not supported"

    # q path on host: phiq = elu(x@Wq + bq) + 1
    q = x @ Wq + bq
    phiq = np.where(q > 0, q + 1.0, np.exp(np.minimum(q, 0.0))).astype(np.float32)

    bf = ml_dtypes.bfloat16
    f8 = mybir.dt.np(mybir.dt.float8e4)
    Wk32 = Wk * WKSCALE
    wk_p = np.ascontiguousarray(
        Wk32.reshape(NCC, 2, 128, D).transpose(2, 0, 1, 3)).astype(f8)
    wv_p = np.ascontiguousarray(
        Wv.reshape(NDC, 128, D).transpose(1, 0, 2)).astype(bf)
    wo_p = np.ascontiguousarray(
        Wo.reshape(NDC, 128, D).transpose(1, 0, 2)).astype(bf)

    nc = bacc.Bacc("TRN2", target_bir_lowering=False, debug=False,
                   num_devices=NCORES)
    _build(nc, repeat=repeat)
    nc.compile()

    in_maps = []
    for i in range(NCORES):
        sl = slice(i * BPC, (i + 1) * BPC)
        ysl = y[sl]
        yt = ysl.transpose(0, 2, 1)  # [BPC, D, S]
        yk = np.ascontiguousarray(
            yt.reshape(BPC, NCC, 2, 128, NSC, SCHUNK)
            .transpose(0, 4, 3, 1, 2, 5)).astype(f8)
        m = {
            "yk": yk,
            "ysd": ysl.astype(bf),
            "phiqr": np.ascontiguousarray(
                np.broadcast_to(phiq[sl][:, None, :], (BPC, 128, D))).astype(bf),
            "wk": wk_p,
            "wv": wv_p,
            "wo": wo_p,
        }
        in_maps.append(m)
    return nc, in_maps


def run(inputs, trace=False):
    """Build, compile, and execute on 8 NeuronCores. Returns (out, results)."""
    nc, in_maps = prepare(inputs)
    res = run_bass_kernel_spmd(nc, in_maps, list(range(NCORES)), trace=trace)
    bo = np.asarray(inputs["bo"], np.float32).reshape(1, D)
    out = np.concatenate([r["out"] for r in res.results], axis=0) + bo
    return np.ascontiguousarray(out.astype(np.float32)), res


def kernel(**inputs) -> np.ndarray:
    out, _ = run(inputs, trace=False)
    return out


# revision 18
# speedup vs baseline: 1.0681x; 1.0455x over previous
"""Trainium2 Bass kernel for ContextualGatingCollapse (linear attention, one query per batch).

Math (per batch b, head h, phi(z) = elu(z)+1 = min(exp(z),1) + relu(z)):
    phiq = phi(x @ Wq + bq)                     [1, 1024]   (host)
    k    = y @ Wk                               [S, 1024]   (PE, fp8 DoubleRow)
    w[s,h] = sum_{d in block_h} phi(k)[s,d] * phiq[d]       (ACT exp/relu + DVE stt/mul/reduce)
    u[h,:] = sum_s w[s,h] * y[s,:]              [16, 1024]  (PE, fp32 PSUM)
    den[h] = sum_s w[s,h]                                    (PE, ones column in the u-acc matmul)
    ctx[block_h] = (u[h,:] / (den[h]+eps)) @ Wv[:, block_h]  (PE finale)
    out  = ctx @ Wo + bo                                     (PE finale + host bias)

Never computes the V projection: each head's numerator only reads head-block
columns of V, so num[h, block_h] = (sum_s w[s,h] y[s,:]) @ Wv[:, block_h].
The q path runs on host.

Engine balance per 128-row subtile (measured-model ns, both near-saturated):
    PE   : 8 DR matmuls (kp) + 3 deferred u/den matmuls    ~2800
    ACT  : exp + relu from PSUM                            ~2100
    DVE  : stt combine + phiq mul + per-head reduce        ~2900
GPSIMD/Pool offload was tried and is a dead end: its bf16 tensor ops run far
below the DVE on real hardware.  fp16 for the u-acc matmul operands was also
tried: fp16 matmuls are slower than bf16 on the PE.

K projection dtype: fp8-e4m3 with DoubleRow (2 MACs/cell/cycle).  Wk is
pre-scaled by 32 (exact power of 2) so fp8 operands have unit variance; the
ACT affine (scale=1/32) undoes it before exp/relu.  All accumulation fp32 PSUM.

Sharding: data-parallel over batch, 2 batches per NeuronCore x 8 cores.
"""

import os
import sys

import numpy as np

for _p in ("/opt/trn_rl_repo", "/root/.axon_site/_ro/trn_rl_repo"):
    if os.path.isdir(_p) and _p not in sys.path:
        sys.path.insert(0, _p)

import ml_dtypes
from contextlib import ExitStack

import concourse.bass as bass
import concourse.tile as tile
from concourse import bacc, mybir
from concourse.bass_utils import run_bass_kernel_spmd

B, S, D, H, HD = 16, 4096, 1024, 16, 64
NCORES = 8
BPC = B // NCORES      # 2 batches per core
EPS = 1e-6
FP = mybir.dt.float32
BF = mybir.dt.bfloat16
F16 = mybir.dt.float16
F8 = mybir.dt.float8e4
SCHUNK = 512           # s-chunk per DMA tile
NSC = S // SCHUNK      # 8
NSUB = SCHUNK // 128   # 4 psum subtiles per s-chunk
NDC = D // 128         # 8 contraction chunks (bf16)
NCC = D // 256         # 4 contraction chunks (fp8 DoubleRow)
WKSCALE = 32.0
UDELAY = 4             # u-acc deferral depth (subtiles)
Exp = mybir.ActivationFunctionType.Exp
Relu = mybir.ActivationFunctionType.Relu
Add = mybir.AluOpType.add
Min = mybir.AluOpType.min
Mult = mybir.AluOpType.mult
AxX = mybir.AxisListType.X
AxC = mybir.AxisListType.C
DR = mybir.MatmulPerfMode.DoubleRow


def _emit_u(nc, acc, onescol_f16, pending, last):
    """u/den accumulation for one (deferred) subtile: u += w0r.T @ y_sd,
    den += w0r.T @ ones.  Emitted UDELAY subtiles late so the PE never waits
    on the phi chain."""
    w0r, ys, j, first = pending
    nc.tensor.matmul(acc[:, 0:512], lhsT=w0r, rhs=ys[:, j, 0:512],
                     start=first, stop=last)
    nc.tensor.matmul(acc[:, 512:1024], lhsT=w0r, rhs=ys[:, j, 512:1024],
                     start=first, stop=last)
    nc.tensor.matmul(acc[:, 1024:1026], lhsT=w0r, rhs=onescol_f16[:],
                     start=first, stop=last)


def _build(nc: bass.Bass, repeat: int = 1):
    yk_d = nc.dram_tensor("yk", [BPC, NSC, 128, NCC, 2, SCHUNK], F8,
                          kind="ExternalInput")
    wk_d = nc.dram_tensor("wk", [128, NCC, 2, D], F8, kind="ExternalInput")
    ysd_d = nc.dram_tensor("ysd", [BPC, S, D], BF, kind="ExternalInput")
    phiq_d = nc.dram_tensor("phiqr", [BPC, 128, 2 * D], BF, kind="ExternalInput")
    wv_d = nc.dram_tensor("wv", [128, NDC, D], BF, kind="ExternalInput")
    wo_d = nc.dram_tensor("wo", [128, NDC, D], BF, kind="ExternalInput")
    out_d = nc.dram_tensor("out", [BPC, D], FP, kind="ExternalOutput")

    # Small constants embedded in the NEFF.
    onescol_d = nc.inline_tensor(np.ones((128, 2), np.float32), "onescol")
    eye16_d = nc.inline_tensor(np.eye(H, dtype=np.float32), "eye16")
    ident1_d = nc.inline_tensor(np.ones((1, 1), np.float32), "ident1")
    ones16_d = nc.inline_tensor(np.ones((H, 1), np.float32), "ones16")
    mask_np = np.zeros((H, D), np.float32)
    for h in range(H):
        mask_np[h, h * HD:(h + 1) * HD] = 1.0
    mask_d = nc.inline_tensor(mask_np, "maskhd")

    with tile.TileContext(nc) as tc, ExitStack() as ctx:
        wpool = ctx.enter_context(tc.tile_pool(name="wpool", bufs=1))
        cpool = ctx.enter_context(tc.tile_pool(name="cpool", bufs=1))
        # PSUM: 2 rotating kp tiles (2 banks each) + one 3-bank accumulator
        # (u in cols 0..1023, den in 1024..1025).
        kpool = ctx.enter_context(
            tc.tile_pool(name="kpool", bufs=2, space=bass.MemorySpace.PSUM))
        accps = ctx.enter_context(
            tc.tile_pool(name="accps", bufs=1, space=bass.MemorySpace.PSUM))

        # ---- weights / constants needed by the main loop ----
        wk_sb = wpool.tile([128, NCC, 2, D], F8, tag="wk")
        nc.sync.dma_start(wk_sb[:], wk_d[:])
        phiq_sb = wpool.tile([128, BPC, 2 * D], BF, tag="phiq")
        nc.sync.dma_start(phiq_sb[:], phiq_d[:].rearrange("b p d -> p b d"))

        # One-time all-engine sync so steady-state instructions don't carry
        # per-weight-DMA waits (walrus caps sync-wait commands per inst).
        tc.strict_bb_all_engine_barrier()

        # Finale-only tensors: issued after the barrier so they load during
        # the main loop instead of lengthening startup.
        wv_sb = wpool.tile([128, NDC, D], BF, tag="wv")
        nc.sync.dma_start(wv_sb[:], wv_d[:])
        wo_sb = wpool.tile([128, NDC, D], BF, tag="wo")
        nc.sync.dma_start(wo_sb[:], wo_d[:])
        mask_sb = cpool.tile([H, D], FP, tag="mask")
        nc.sync.dma_start(mask_sb[:], mask_d[:])
        eye16_sb = cpool.tile([H, H], FP, tag="eye16")
        nc.sync.dma_start(eye16_sb[:], eye16_d[:])
        ident1_sb = cpool.tile([1, 1], FP, tag="ident1")
        nc.sync.dma_start(ident1_sb[:], ident1_d[:])
        ones16_sb = cpool.tile([H, 1], FP, tag="ones16")
        nc.sync.dma_start(ones16_sb[:], ones16_d[:])
        ones16_bf = cpool.tile([H, 1], BF, tag="ones16bf")
        nc.vector.tensor_copy(ones16_bf[:], ones16_sb[:])
        onescol_sb = cpool.tile([128, 2], FP, tag="onescol")
        nc.sync.dma_start(onescol_sb[:], onescol_d[:])
        onescol_f16 = cpool.tile([128, 2], BF, tag="onescolf16")
        nc.vector.tensor_copy(onescol_f16[:], onescol_sb[:])

        ykpool = ctx.enter_context(tc.tile_pool(name="ykpool", bufs=3))
        yspool = ctx.enter_context(tc.tile_pool(name="yspool", bufs=3))
        work = ctx.enter_context(tc.tile_pool(name="work", bufs=2))
        lpool = ctx.enter_context(tc.tile_pool(name="lpool", bufs=UDELAY + 2))
        upool = ctx.enter_context(tc.tile_pool(name="upool", bufs=1))

        # Optional in-kernel repetition (timing only).
        rep_cm = tc.For_i(0, repeat, 1) if repeat > 1 else None
        if rep_cm is not None:
            rep_cm.__enter__()

        u_sb = [upool.tile([H, D + 2], FP, tag=f"u{b}", name=f"u{b}")
                for b in range(BPC)]

        for b in range(BPC):
            acc = accps.tile([H, D + 2], FP, tag="acc")
            pend = []
            sub = 0
            for sc in range(NSC):
                yk = ykpool.tile([128, NCC, 2, SCHUNK], F8, tag="yk")
                nc.sync.dma_start(
                    yk[:], yk_d[b:b + 1, sc:sc + 1].rearrange(
                        "o t p c i s -> (o t p) c i s"))
                ys = yspool.tile([128, NSUB, D], BF, tag="ys")
                nc.sync.dma_start(
                    ys[:], ysd_d[b:b + 1, sc * SCHUNK:(sc + 1) * SCHUNK, :]
                    .rearrange("o (j p) d -> (o p) j d", p=128))
                for jp in range(NSUB // 2):
                    # phi(k) = min(exp(k),1) + relu(k); ACT affine undoes the
                    # host-side Wk*32 scaling.  ACT drains each kp PSUM tile
                    # per subtile into halves of [128, 2D] SBUF tiles; the
                    # DVE stt/mul/reduce then run once per subtile PAIR,
                    # amortising fixed per-op cost (stt/reduce are 1x ops).
                    e_t = work.tile([128, 2 * D], BF, tag="e")
                    r_t = work.tile([128, 2 * D], BF, tag="r")
                    for jj in range(2):
                        j = 2 * jp + jj
                        kp = kpool.tile([128, D], FP, tag="kp")
                        for cc in range(NCC):
                            fl, ll = cc == 0, cc == NCC - 1
                            lt = yk[:, cc, :, bass.ts(j, 128)]
                            nc.tensor.matmul(kp[:, 0:512], lhsT=lt,
                                             rhs=wk_sb[:, cc, :, 0:512],
                                             start=fl, stop=ll, perf_mode=DR)
                            nc.tensor.matmul(kp[:, 512:1024], lhsT=lt,
                                             rhs=wk_sb[:, cc, :, 512:1024],
                                             start=fl, stop=ll, perf_mode=DR)
                        nc.scalar.activation(e_t[:, bass.ts(jj, D)], kp[:],
                                             Exp, scale=1.0 / WKSCALE)
                        nc.scalar.activation(r_t[:, bass.ts(jj, D)], kp[:],
                                             Relu, scale=1.0 / WKSCALE)
                    pk = work.tile([128, 2 * D], BF, tag="pk")
                    nc.vector.scalar_tensor_tensor(pk[:], e_t[:], 1.0, r_t[:],
                                                   Min, Add)
                    pkq = work.tile([128, 2 * D], BF, tag="pkq")
                    nc.vector.tensor_mul(pkq[:], pk[:], phiq_sb[:, b, :])
                    w0r = lpool.tile([128, 2 * H], BF, tag="w0r")
                    with nc.allow_low_precision(reason="single rounding to "
                                                "bf16 for the u-acc lhsT"):
                        nc.vector.tensor_reduce(
                            w0r[:], pkq[:].rearrange("p (h d) -> p h d", h=2 * H),
                            axis=AxX, op=Add)
                    for jj in range(2):
                        j = 2 * jp + jj
                        pend.append((w0r[:, bass.ts(jj, H)], ys, j, sub == 0))
                        if len(pend) > UDELAY:
                            _emit_u(nc, acc, onescol_f16, pend.pop(0),
                                    last=False)
                        sub += 1
            for i, p in enumerate(pend):
                _emit_u(nc, acc, onescol_f16, p, last=(i == len(pend) - 1))
            pend = []
            nc.vector.tensor_copy(u_sb[b][:], acc[:])

        # ---- finale: ctx[block_h] = (u[h,:] @ Wv[:, block_h]) / den[h],
        # out = ctx @ Wo.  The 1/den row scale commutes through Wv, so raw u
        # is transposed (PE) and rcp folds into the mask stt on the drain.
        usT = cpool.tile([128, NDC, BPC, H], BF, tag="usT")
        ctx_rows = [cpool.tile([1, D], FP, tag=f"ctxrow{b}", name=f"ctxrow{b}")
                    for b in range(BPC)]
        for b in range(BPC):
            for c in range(NDC):
                tp = kpool.tile([128, H], FP, tag="kp")
                nc.tensor.transpose(tp[:], u_sb[b][:, bass.ts(c, 128)],
                                    eye16_sb[:])
                nc.scalar.copy(usT[:, c, b, :], tp[:])
            dsb = cpool.tile([H, 1], FP, tag=f"dsb{b}")
            nc.vector.tensor_scalar_add(dsb[:], u_sb[b][:, D:D + 1], EPS)
            rcp = cpool.tile([H, 1], FP, tag=f"rcp{b}")
            nc.vector.reciprocal(rcp[:], dsb[:])
            cs = accps.tile([H, D], FP, tag="acc")
            for n in range(2):
                for c in range(NDC):
                    nc.tensor.matmul(cs[:, bass.ts(n, 512)],
                                     lhsT=usT[:, c, b, :],
                                     rhs=wv_sb[:, c, bass.ts(n, 512)],
                                     start=(c == 0), stop=(c == NDC - 1))
            # (cs * 1/den) * head-block mask, fused in one DVE op
            csm = cpool.tile([H, D], BF, tag=f"csm{b}")
            nc.vector.scalar_tensor_tensor(csm[:], cs[:], rcp[:], mask_sb[:],
                                           Mult, Mult)
            for n in range(2):
                cr = kpool.tile([1, 512], FP, tag="kp")
                nc.tensor.matmul(cr[:], lhsT=ones16_bf[:],
                                 rhs=csm[:, bass.ts(n, 512)],
                                 start=True, stop=True)
                nc.scalar.copy(ctx_rows[b][:, bass.ts(n, 512)], cr[:])
        ctxT = cpool.tile([128, NDC, BPC], BF, tag="ctxT")
        for c in range(NDC):
            for b in range(BPC):
                tp = kpool.tile([128, 1], FP, tag="kp")
                nc.tensor.transpose(tp[:], ctx_rows[b][:, bass.ts(c, 128)],
                                    ident1_sb[:])
                nc.scalar.copy(ctxT[:, c, b:b + 1], tp[:])
        out_sb = cpool.tile([BPC, D], FP, tag="outsb")
        for n in range(2):
            op = accps.tile([BPC, 512], FP, tag="acc")
            for c in range(NDC):
                nc.tensor.matmul(op[:], lhsT=ctxT[:, c, :],
                                 rhs=wo_sb[:, c, bass.ts(n, 512)],
                                 start=(c == 0), stop=(c == NDC - 1))
            nc.scalar.copy(out_sb[:, bass.ts(n, 512)], op[:])
        nc.sync.dma_start(out_d[:], out_sb[:])

        if rep_cm is not None:
            rep_cm.__exit__(None, None, None)

    return nc


def prepare(inputs, repeat: int = 1):
    """Build + bacc-compile the program and the per-core input maps."""
    y = np.asarray(inputs["y_superposed"], np.float32)
    x = np.asarray(inputs["x_context"], np.float32)
    Wq = np.asarray(inputs["Wq"], np.float32)
    bq = np.asarray(inputs["bq"], np.float32).reshape(1, D)
    Wk = np.asarray(inputs["Wk"], np.float32)
    bk = np.asarray(inputs["bk"], np.float32)
    Wv = np.asarray(inputs["Wv"], np.float32)
    bv = np.asarray(inputs["bv"], np.float32)
    Wo = np.asarray(inputs["Wo"], np.float32)
    bo = np.asarray(inputs["bo"], np.float32).reshape(1, D)
    assert not np.any(bk) and not np.any(bv), "nonzero bk/bv not supported"

    # q path on host: phiq = elu(x@Wq + bq) + 1
    q = x @ Wq + bq
    phiq = np.where(q > 0, q + 1.0, np.exp(np.minimum(q, 0.0))).astype(np.float32)

    bf = ml_dtypes.bfloat16
    f8 = mybir.dt.np(mybir.dt.float8e4)
    Wk32 = Wk * WKSCALE
    wk_p = np.ascontiguousarray(
        Wk32.reshape(NCC, 2, 128, D).transpose(2, 0, 1, 3)).astype(f8)
    wv_p = np.ascontiguousarray(
        Wv.reshape(NDC, 128, D).transpose(1, 0, 2)).astype(bf)
    wo_p = np.ascontiguousarray(
        Wo.reshape(NDC, 128, D).transpose(1, 0, 2)).astype(bf)

    nc = bacc.Bacc("TRN2", target_bir_lowering=False, debug=False,
                   num_devices=NCORES)
    _build(nc, repeat=repeat)
    nc.compile()

    in_maps = []
    for i in range(NCORES):
        sl = slice(i * BPC, (i + 1) * BPC)
        ysl = y[sl]
        yt = ysl.transpose(0, 2, 1)  # [BPC, D, S]
        yk = np.ascontiguousarray(
            yt.reshape(BPC, NCC, 2, 128, NSC, SCHUNK)
            .transpose(0, 4, 3, 1, 2, 5)).astype(f8)
        m = {
            "yk": yk,
            "ysd": ysl.astype(bf),
            "phiqr": np.ascontiguousarray(np.broadcast_to(
                np.concatenate([phiq[sl], phiq[sl]], axis=-1)[:, None, :],
                (BPC, 128, 2 * D))).astype(bf),
            "wk": wk_p,
            "wv": wv_p,
            "wo": wo_p,
        }
        in_maps.append(m)
    return nc, in_maps


def run(inputs, trace=False):
    """Build, compile, and execute on 8 NeuronCores. Returns (out, results)."""
    nc, in_maps = prepare(inputs)
    res = run_bass_kernel_spmd(nc, in_maps, list(range(NCORES)), trace=trace)
    bo = np.asarray(inputs["bo"], np.float32).reshape(1, D)
    out = np.concatenate([r["out"] for r in res.results], axis=0) + bo
    return np.ascontiguousarray(out.astype(np.float32)), res


def kernel(**inputs) -> np.ndarray:
    out, _ = run(inputs, trace=False)
    return out
